# revision 6
# baseline (speedup 1.0000x reference)
"""Trainium2 Bass kernel for segmented LogSumExp over per-image cell logits.

For image i with n_i cells (contiguous rows of cell_logits):
    out_i = (1/R) * (logsumexp(R * x_i, axis=0) - log(n_i)),  R = 5.0
Empty images produce zero rows.

Strategy (data-parallel over 8 NeuronCores, no cross-core communication):
  * Host sorts non-empty images by cell count (desc) and packs them into
    blocks of 32 (= 8 cores x 4 image-lanes). All images in a block are
    padded to the block max count rounded up to a multiple of GRAN, so
    every core sees the *identical* segment structure (SPMD: one program).
  * Per-core device layout: [128, F] f32 where partition p = 32*j + class
    (j = image-lane 0..3), free axis = padded cells, images back to back.
    Padding cells hold -1e30 so exp(R*x) -> 0.
  * Device: DMA ~1MB chunks -> ACT exp(scale=R) in place -> DVE segmented
    tensor_reduce (fixed segment length per run of equal-size blocks) into
    an accumulator [128, nblocks] -> ACT Ln -> DMA out.
  * Host applies (lnS - log n)/R and scatters back to [B, 32].
"""
import numpy as np

R = 5.0
C = 32
N_CORES = 8
LANES = 4                      # images per core per block (4*32 classes = 128 partitions)
IMGS_PER_BLOCK = N_CORES * LANES
GRAN = 2                       # segment lengths padded to a multiple of this
L_TARGET = 2048                # target chunk width (cols) ~= 1MB per [128, L] f32 tile
PAD_VAL = np.float32(-1e30)


def _plan(counts):
    """Pack images into blocks; return layout metadata (identical across cores)."""
    B = counts.shape[0]
    offsets = np.zeros(B, np.int64)
    np.cumsum(counts[:-1], out=offsets[1:])

    nz = np.nonzero(counts > 0)[0]
    order = nz[np.argsort(-counts[nz], kind="stable")]
    n_img = order.shape[0]
    if n_img == 0:
        return None
    nblocks = -(-n_img // IMGS_PER_BLOCK)
    order_p = np.concatenate(
        [order, np.full(nblocks * IMGS_PER_BLOCK - n_img, -1, np.int64)]
    )
    grid = order_p.reshape(nblocks, N_CORES, LANES)        # [p, c, j] image ids
    blockmax = counts[grid[:, 0, 0]]                       # sorted desc -> first is max
    M_blocks = (-(-blockmax // GRAN) * GRAN).astype(np.int64)  # padded segment length
    S = np.zeros(nblocks + 1, np.int64)
    np.cumsum(M_blocks, out=S[1:])
    F = int(S[-1])

    # runs of consecutive blocks with equal M -> one tensor_reduce per (run, chunk)
    runs = []                                              # (M, first_block, nblocks_in_run)
    i = 0
    while i < nblocks:
        j = i
        while j < nblocks and M_blocks[j] == M_blocks[i]:
            j += 1
        runs.append((int(M_blocks[i]), i, j - i))
        i = j

    # chunks: [col0, L, [(tile_off, acc_col0, m, M), ...]] cut only at segment bounds
    chunks = []
    cur_col0, cur_len, cur_ops = 0, 0, []
    for (M, b0, nb_run) in runs:
        done = 0
        while done < nb_run:
            room = max(L_TARGET - cur_len, 0)
            take = min(nb_run - done, room // M)
            if take == 0:
                if cur_len > 0:
                    chunks.append((cur_col0, cur_len, cur_ops))
                    cur_col0, cur_len, cur_ops = cur_col0 + cur_len, 0, []
                take = min(nb_run - done, max(L_TARGET // M, 1))
            cur_ops.append((cur_len, b0 + done, take, M))
            cur_len += take * M
            done += take
    if cur_len > 0:
        chunks.append((cur_col0, cur_len, cur_ops))
    # merge a tiny tail chunk into its predecessor to keep DMAs big
    if len(chunks) >= 2 and chunks[-1][1] < 512:
        (c0a, la, opsa), (c0b, lb, opsb) = chunks[-2], chunks[-1]
        chunks[-2:] = [(c0a, la + lb, opsa + [(la + t, q, m, M) for (t, q, m, M) in opsb])]

    return dict(offsets=offsets, grid=grid, M_blocks=M_blocks, S=S, F=F,
                nblocks=nblocks, runs=runs, chunks=chunks)


def _build_inputs(x, counts, plan):
    """Per-core [128, F] arrays via one flat gather (f32)."""
    N = x.shape[0]
    F, S, grid = plan["F"], plan["S"], plan["grid"]
    M_blocks, nblocks, offsets = plan["M_blocks"], plan["nblocks"], plan["offsets"]

    col_p = np.repeat(np.arange(nblocks, dtype=np.int32), M_blocks)      # [F]
    col_f = (np.arange(F, dtype=np.int64) - np.repeat(S[:-1], M_blocks)) # [F]

    img = grid[col_p].transpose(1, 2, 0)                   # [cores, lanes, F]
    img_c = np.clip(img, 0, None)
    valid = (img >= 0) & (col_f[None, None, :] < counts[img_c])
    # row N: PAD_VAL poison row; row N+1: zeros row (lane-pad images get one
    # zero cell at f==0 so their segment sum is 1 -> Ln = 0, no infs on device)
    row = np.where(valid, offsets[img_c] + col_f[None, None, :], N)
    row[(img < 0) & (col_f[None, None, :] == 0)] = N + 1
    x_ext = np.vstack([x, np.full((1, C), PAD_VAL, np.float32), np.zeros((1, C), np.float32)])

    idx = (row.astype(np.int64) * C)[:, :, None, :] + np.arange(C, dtype=np.int64)[None, None, :, None]
    X_all = x_ext.ravel()[idx]                             # [cores, lanes, C, F]
    return np.ascontiguousarray(X_all.reshape(N_CORES, 128, F))


def _build_program(F, nb, chunks, reps=1):
    from contextlib import ExitStack
    import concourse.tile as tile
    from concourse import bacc, mybir

    nc = bacc.Bacc("TRN2", debug=False, num_devices=N_CORES)
    x_ap = nc.dram_tensor("xdata", [128, F], mybir.dt.float32, kind="ExternalInput").ap()
    out_ap = nc.dram_tensor("out", [128, nb], mybir.dt.float32, kind="ExternalOutput").ap()

    with tile.TileContext(nc) as tc, ExitStack() as ctx:
        pool = ctx.enter_context(tc.tile_pool(name="chunks", bufs=8))
        accp = ctx.enter_context(tc.tile_pool(name="acc", bufs=2))
        if reps == 0:  # timing baseline: in/out DMA only
            t0 = accp.tile([128, nb], mybir.dt.float32, tag="ot")
            nc.sync.dma_start(t0[:], x_ap[:, 0:nb])
            nc.sync.dma_start(out_ap[:], t0[:])
        for rep in range(reps):
            acc = accp.tile([128, nb], mybir.dt.float32, tag="acc")
            for (col0, L, ops) in chunks:
                t = pool.tile([128, L], mybir.dt.float32, tag="chunk")
                nc.sync.dma_start(t[:], x_ap[:, col0:col0 + L])
                nc.scalar.activation(t[:], t[:], mybir.ActivationFunctionType.Exp, scale=R)
                for (toff, q0, m, M) in ops:
                    nc.vector.tensor_reduce(
                        acc[:, q0:q0 + m],
                        t[:, toff:toff + m * M].rearrange("p (m k) -> p m k", k=M),
                        axis=mybir.AxisListType.X,
                        op=mybir.AluOpType.add,
                    )
            if rep == reps - 1:
                ot = accp.tile([128, nb], mybir.dt.float32, tag="ot")
                nc.scalar.activation(ot[:], acc[:], mybir.ActivationFunctionType.Ln)
                nc.sync.dma_start(out_ap[:], ot[:])
    nc.compile()
    return nc


def kernel(cell_logits, cell_counts, _reps=1):
    x = np.asarray(cell_logits, dtype=np.float32)
    counts = np.asarray(cell_counts).astype(np.int64)
    B = counts.shape[0]
    out = np.zeros((B, C), dtype=np.float32)

    plan = _plan(counts)
    if plan is None:
        return out

    X_all = _build_inputs(x, counts, plan)
    nb = plan["nblocks"]

    nc = _build_program(plan["F"], nb, plan["chunks"], reps=_reps)

    from concourse.bass_utils import run_bass_kernel_spmd
    res = run_bass_kernel_spmd(
        nc, [{"xdata": X_all[c]} for c in range(N_CORES)], list(range(N_CORES))
    )
    lnS = np.stack([res.results[c]["out"] for c in range(N_CORES)])  # [cores, 128, nb]
    lnS = lnS.reshape(N_CORES, LANES, C, nb)

    grid = plan["grid"]                                    # [p, c, j]
    imgs_flat = grid.transpose(1, 2, 0).ravel()            # [cores*lanes*nb]
    n_flat = counts[np.clip(grid, 0, None)].transpose(1, 2, 0).ravel()
    vals = lnS.transpose(0, 1, 3, 2).reshape(-1, C) / R    # [cores*lanes*nb, C]
    mask = imgs_flat >= 0
    vals = vals[mask] - (np.log(n_flat[mask].astype(np.float64)) / R)[:, None].astype(np.float32)
    out[imgs_flat[mask]] = vals
    return out
